# revision 26
# baseline (speedup 1.0000x reference)
"""Multi-head self-attention (B=8, N=1024, C=768, H=12, D=64) on 8 Trainium2
NeuronCores, batch-parallel (one batch element per core).

Per-core dataflow (activations kept feature-major, "T" = [feature, token]):
  xT [768,1024] --(PE)--> QT,KT [768,1024] (d-major) and V [1024,768+ones]
  S^T[k,q] = KT_h-slice^T x QT_h      (K=d=64; two heads of a pair via
                                       PE row-tiling at partitions 0/64)
  E = exp(S^T * scale) -> bf16        (ACT; no max-subtract: |S*scale| < 9)
  ctxU^T[d,q] (+denominator row) = V_ext_h^T x E   (ones column in V gives
                                                    the softmax denominator)
  evac ctxU fast (frees PSUM); one pair behind: reciprocal + masked K=1
  ones-matmul broadcast + one in-place multiply normalizes the pair.
  out[q,o] = ctxN^T-slices^T x wpT + bias(bcast, DVE add)

Matmuls run float32r (fp22 multiply, fp32 accumulate, 2 PE cycles/col)
except the PV stage, whose operands (V, E) are bf16 (1 cycle/col): softmax
weights are positive so bf16 there adds no cancellation-amplified error.
"""
import numpy as np

import concourse.bass as bass
import concourse.tile as tile
from concourse import bacc, mybir
from concourse.bass_utils import run_bass_kernel_spmd

N_CORES = 8
N = 1024          # tokens per core (batch element)
C = 768           # model dim
H = 12            # heads
D = 64            # head dim
SCALE = D ** -0.5
NT = N // 128     # 8 token tiles
CT = C // 128     # 6 feature tiles
F32 = mybir.dt.float32
F32R = mybir.dt.float32r
BF16 = mybir.dt.bfloat16
EXP = mybir.ActivationFunctionType.Exp

QK_BF16 = False   # False: keep the Q/K path (C-phase + S matmuls) in fp32r


def _r(ap):
    return ap.bitcast(F32R)


def build():
    nc = bacc.Bacc(
        "TRN2", target_bir_lowering=False, debug=False, num_devices=N_CORES
    )
    xT_d = nc.dram_tensor("xT", [C, N], F32, kind="ExternalInput").ap()
    wqT_d = nc.dram_tensor("wqT", [C, 3 * C], F32, kind="ExternalInput").ap()
    wpT_d = nc.dram_tensor("wpT", [C, C], F32, kind="ExternalInput").ap()
    bias_d = nc.dram_tensor("bias_bc", [128, C], F32, kind="ExternalInput").ap()
    ones_d = nc.dram_tensor("ones_v", [128, H], F32, kind="ExternalInput").ap()
    onesr_d = nc.dram_tensor("ones_mask", [2, 128], F32, kind="ExternalInput").ap()
    out_d = nc.dram_tensor("out", [N, C], F32, kind="ExternalOutput").ap()

    qk_dt = BF16 if QK_BF16 else F32
    qk = (lambda ap: ap) if QK_BF16 else _r

    with tile.TileContext(nc) as tc:
        with (
            tc.tile_pool(name="big", bufs=1) as big,
            tc.tile_pool(name="wqk", bufs=12) as wqkp,
            tc.tile_pool(name="e", bufs=4) as ep,
            tc.tile_pool(name="outb", bufs=2) as outp,
            tc.tile_pool(name="norm", bufs=2) as normp,
            tc.tile_pool(name="psA", bufs=2, space="PSUM") as psA,
            tc.tile_pool(name="psC", bufs=2, space="PSUM") as psC,
        ):
            # ---- persistent SBUF tensors -------------------------------
            xqk = big.tile([128, CT, N], F32, name="xqk", tag="xqk")
            wvs = big.tile([128, CT, C], F32, name="wvs", tag="wvs")
            wps = big.tile([128, CT, C], BF16, name="wps", tag="wps")
            QT = big.tile([128, CT, N], qk_dt, name="QT", tag="QT")
            KT = big.tile([128, CT, N], qk_dt, name="KT", tag="KT")
            V = big.tile([128, NT, H * (D + 1)], BF16, name="V", tag="V")
            ctxN = big.tile([128, CT, N], BF16, name="ctxN", tag="ctxN")
            bias_sb = big.tile([128, C], F32, name="bias_sb", tag="bias")
            ones_mask = [
                big.tile([1, 128], F32, name=f"ones_mask{i}", tag=f"onesr{i}")
                for i in range(2)
            ]

            nc.sync.dma_start(bias_sb[:], bias_d[:])
            for i in range(2):
                nc.sync.dma_start(_r(ones_mask[i][:]), _r(onesr_d[i:i + 1, :]))
            for ct in range(CT):
                nc.sync.dma_start(
                    _r(xqk[:, ct, :]), _r(xT_d[ct * 128:(ct + 1) * 128, :])
                )
                nc.sync.dma_start(
                    _r(wvs[:, ct, :]), _r(wqT_d[ct * 128:(ct + 1) * 128, 2 * C:3 * C])
                )
            for nt in range(NT):
                vt = V[:, nt, :].rearrange("p (h e) -> p h e", e=D + 1)
                nc.gpsimd.dma_start(
                    vt[:, :, D:D + 1], ones_d.rearrange("p (h o) -> p h o", o=1)
                )

            # ---- phase B: V (token-major, bf16) ------------------------
            for nt in range(NT):
                pv = psA.tile([128, N], F32, tag="ps", name=f"pv{nt}")
                for ct in range(CT):
                    lhsT = xqk[:, ct, nt * 128:(nt + 1) * 128]
                    for lo, w in ((0, 512), (512, 256)):
                        nc.tensor.matmul(
                            pv[:, lo:lo + w],
                            _r(lhsT),
                            _r(wvs[:, ct, lo:lo + w]),
                            start=(ct == 0),
                            stop=(ct == CT - 1),
                        )
                vt = V[:, nt, :].rearrange("p (h e) -> p h e", e=D + 1)
                nc.scalar.copy(
                    vt[:, :, 0:D], pv[:, 0:C].rearrange("p (h d) -> p h d", d=D)
                )

            # ---- phase C: QT / KT (feature-major) ----------------------
            for jt in range(CT):
                for base, dst in ((0, QT), (C, KT)):
                    wts = []
                    for ct in range(CT):
                        wt = wqkp.tile(
                            [128, 128], qk_dt, tag="wqk", name=f"w{base}_{jt}_{ct}"
                        )
                        src = wqT_d[
                            ct * 128:(ct + 1) * 128,
                            base + jt * 128:base + (jt + 1) * 128,
                        ]
                        if QK_BF16:
                            nc.gpsimd.dma_start(wt[:], src)
                        else:
                            nc.sync.dma_start(_r(wt[:]), _r(src))
                        wts.append(wt)
                    ps = psA.tile([128, N], F32, tag="ps", name=f"q{base}_{jt}")
                    for ct in range(CT):
                        for qc in range(2):
                            nc.tensor.matmul(
                                ps[:, qc * 512:(qc + 1) * 512],
                                qk(wts[ct][:]),
                                qk(xqk[:, ct, qc * 512:(qc + 1) * 512]),
                                start=(ct == 0),
                                stop=(ct == CT - 1),
                            )
                    nc.scalar.copy(qk(dst[:, jt, :]), ps[:])

            # proj weights are first needed far later; load them now so the
            # casting DMAs do not delay the startup x/w loads
            for ct in range(CT):
                nc.gpsimd.dma_start(wps[:, ct, :], wpT_d[ct * 128:(ct + 1) * 128, :])

            # ---- phase D: attention, head pairs, row-packed S ----------
            deferred_norm = []

            def emit_norm(jobs):
                # jobs = halves of one or more pairs; per pair, build the
                # full [128, N] reciprocal-broadcast with two K=1 masked
                # ones-matmuls, then normalize with a single multiply.
                for i in range(0, len(jobs), 2):
                    emit_norm_pair(jobs[i:i + 2])

            def emit_norm_pair(jobs, dest=None):
                p_ = jobs[0][2]
                rcrs = []
                for den_, h_, _p in jobs:
                    rc = normp.tile([1, N], F32, tag="rc", name=f"rc{h_}", bufs=2)
                    nc.vector.reciprocal_approx_fast(rc[:], den_[:])
                    rcr = normp.tile([1, N], F32, tag="rcr", name=f"rcr{h_}", bufs=2)
                    nc.scalar.copy(_r(rcr[:]), rc[:])
                    rcrs.append(rcr)
                bc_ps = psA.tile([128, N], F32, tag="ps", name=f"bcp{p_}")
                for qc in range(2):
                    for half, rcr in enumerate(rcrs):
                        nc.tensor.matmul(
                            bc_ps[:, qc * 512:(qc + 1) * 512],
                            _r(ones_mask[half][:]),
                            _r(rcr[:, qc * 512:(qc + 1) * 512]),
                            start=(half == 0),
                            stop=(half == len(rcrs) - 1),
                        )
                bc = normp.tile([128, N], F32, tag="bc", name=f"bc{p_}", bufs=1)
                nc.vector.tensor_copy(bc[:], bc_ps[:])
                dst = ctxN[:, p_, :] if dest is None else dest
                nc.vector.tensor_mul(dst, ctxN[:, p_, :], bc[:])

            # Software pipeline across head pairs: during pair p's S/exp
            # stream (ACT-paced), the PE executes pair p-1's PV matmuls,
            # whose E tiles are already complete. PV then never waits on the
            # in-flight exp, and attention runs at the ACT exp rate.
            def emit_pv(pcps, pes, pp, kt):
                for half in range(2):
                    h = 2 * pp + half
                    for qc in range(2):
                        nc.tensor.matmul(
                            pcps[half][:, qc * 512:(qc + 1) * 512],
                            V[:, kt, h * (D + 1):(h + 1) * (D + 1)],
                            pes[kt][half][:, qc * 512:(qc + 1) * 512],
                            start=(kt == 0),
                            stop=(kt == NT - 1),
                        )

            def emit_evac(pcps, pp):
                for half in range(2):
                    h = 2 * pp + half
                    po = half * 64
                    nc.vector.tensor_copy(
                        ctxN[po:po + 64, pp, :], pcps[half][0:D, :]
                    )
                    den = normp.tile([1, N], F32, tag="den", name=f"den{h}")
                    nc.scalar.copy(den[:], pcps[half][D:D + 1, :])
                    deferred_norm.append((den, h, pp))

            prev = None
            for p in range(CT):  # 6 head pairs; pair p = heads (2p, 2p+1)
                cps = [
                    psC.tile([D + 1, N], F32, tag="ctx", name=f"ctx{2 * p + i}")
                    for i in range(2)
                ]
                es = []
                for kt in range(NT):
                    sps = [
                        psA.tile([128, N], F32, tag="ps", name=f"s{2 * p + i}_{kt}")
                        for i in range(2)
                    ]
                    for half in range(2):
                        po = half * 64
                        for qc in range(2):
                            nc.tensor.matmul(
                                sps[half][:, qc * 512:(qc + 1) * 512],
                                qk(KT[po:po + 64, p, kt * 128:(kt + 1) * 128]),
                                qk(QT[po:po + 64, p, qc * 512:(qc + 1) * 512]),
                                start=True,
                                stop=True,
                                tile_position=(po, 0),
                            )
                    row = []
                    for half in range(2):
                        h = 2 * p + half
                        e = ep.tile(
                            [128, N], BF16, tag="e", name=f"e{h}_{kt}", bufs=12
                        )
                        nc.scalar.activation(e[:], sps[half][:], EXP, scale=SCALE)
                        row.append(e)
                    es.append(row)
                    if prev is not None:
                        emit_pv(prev[0], prev[1], prev[2], kt)
                    if kt == 1 and deferred_norm:
                        # normalize the pair before last while streams run
                        emit_norm(deferred_norm)
                        deferred_norm = []
                if prev is not None:
                    emit_evac(prev[0], prev[2])
                prev = (cps, es, p)
            # drain: PV + evac for the final pair; the second-to-last
            # pair's normalize chain overlaps the drain matmuls
            emit_norm(deferred_norm)
            deferred_norm = []
            for kt in range(NT):
                emit_pv(prev[0], prev[1], prev[2], kt)
            emit_evac(prev[0], prev[2])
            # last pair normalizes into a separate tile: only the ct=5
            # accumulation of each proj chain depends on it
            ctxN5 = big.tile([128, N], BF16, name="ctxN5", tag="ctxN5")
            emit_norm_pair(deferred_norm, dest=ctxN5[:])
            deferred_norm = []

            # ---- phase E: output projection + bias ---------------------
            for nt in range(NT):
                ps = psA.tile([128, N], F32, tag="ps", name=f"po{nt}")
                for lo, w in ((0, 512), (512, 256)):
                    for ct in range(CT):
                        lhs = (
                            ctxN[:, ct, nt * 128:(nt + 1) * 128]
                            if ct < CT - 1
                            else ctxN5[:, nt * 128:(nt + 1) * 128]
                        )
                        nc.tensor.matmul(
                            ps[:, lo:lo + w],
                            lhs,
                            wps[:, ct, lo:lo + w],
                            start=(ct == 0),
                            stop=(ct == CT - 1),
                        )
                ob = outp.tile([128, C], F32, tag="ob", name=f"ob{nt}")
                nc.vector.tensor_add(ob[:], ps[:, 0:C], bias_sb[:])
                nc.sync.dma_start(out_d[nt * 128:(nt + 1) * 128, :], ob[:])

    nc.compile()
    return nc


_CACHE = {}


def _get_nc():
    if "nc" not in _CACHE:
        _CACHE["nc"] = build()
    return _CACHE["nc"]


def run(inputs, trace=False):
    """Run on hardware; returns (full_output [8,1024,768] f32, BassKernelResults)."""
    nc = _get_nc()
    x = np.asarray(inputs["x"], dtype=np.float32)
    w_qkv = np.asarray(inputs["w_qkv"], dtype=np.float32)
    w_proj = np.asarray(inputs["w_proj"], dtype=np.float32)
    b_proj = np.asarray(inputs["b_proj"], dtype=np.float32)

    xT = np.ascontiguousarray(x.transpose(0, 2, 1))          # [8, 768, 1024]
    wqT = np.ascontiguousarray(w_qkv.T)                       # [768, 2304]
    wpT = np.ascontiguousarray(w_proj.T)                      # [768, 768]
    bias_bc = np.ascontiguousarray(np.broadcast_to(b_proj.reshape(1, C), (128, C)))
    ones_v = np.ones((128, H), dtype=np.float32)

    in_maps = [
        {
            "xT": xT[b],
            "wqT": wqT,
            "wpT": wpT,
            "bias_bc": bias_bc,
            "ones_v": ones_v,
            "ones_mask": np.kron(np.eye(2), np.ones((1, 64))).astype(np.float32),
        }
        for b in range(N_CORES)
    ]
    res = run_bass_kernel_spmd(nc, in_maps, list(range(N_CORES)), trace=trace)
    out = np.stack([res.results[b]["out"] for b in range(N_CORES)])
    return out, res


def kernel(x, w_qkv, w_proj, b_proj):
    out, _ = run(
        {"x": x, "w_qkv": w_qkv, "w_proj": w_proj, "b_proj": b_proj}, trace=False
    )
    return out


# revision 27
# speedup vs baseline: 1.0618x; 1.0618x over previous
"""Multi-head self-attention (B=8, N=1024, C=768, H=12, D=64) on 8 Trainium2
NeuronCores, batch-parallel (one batch element per core).

Per-core dataflow (activations kept feature-major, "T" = [feature, token]):
  xT [768,1024] --(PE)--> QT,KT [768,1024] (d-major) and V [1024,768+ones]
  S^T[k,q] = KT_h-slice^T x QT_h      (K=d=64; two heads of a pair via
                                       PE row-tiling at partitions 0/64)
  E = exp(S^T * scale) -> bf16        (ACT; no max-subtract: |S*scale| < 9)
  ctxU^T[d,q] (+denominator row) = V_ext_h^T x E   (ones column in V gives
                                                    the softmax denominator)
  evac ctxU fast (frees PSUM); one pair behind: reciprocal + masked K=1
  ones-matmul broadcast + one in-place multiply normalizes the pair.
  out[q,o] = ctxN^T-slices^T x wpT + bias(bcast, DVE add)

Matmuls run float32r (fp22 multiply, fp32 accumulate, 2 PE cycles/col)
except the PV stage, whose operands (V, E) are bf16 (1 cycle/col): softmax
weights are positive so bf16 there adds no cancellation-amplified error.
"""
import numpy as np

import concourse.bass as bass
import concourse.tile as tile
from concourse import bacc, mybir
from concourse.bass_utils import run_bass_kernel_spmd

N_CORES = 8
N = 1024          # tokens per core (batch element)
C = 768           # model dim
H = 12            # heads
D = 64            # head dim
SCALE = D ** -0.5
NT = N // 128     # 8 token tiles
CT = C // 128     # 6 feature tiles
F32 = mybir.dt.float32
F32R = mybir.dt.float32r
BF16 = mybir.dt.bfloat16
EXP = mybir.ActivationFunctionType.Exp

QK_BF16 = False   # False: keep the Q/K path (C-phase + S matmuls) in fp32r


def _r(ap):
    return ap.bitcast(F32R)


def build():
    nc = bacc.Bacc(
        "TRN2", target_bir_lowering=False, debug=False, num_devices=N_CORES
    )
    xT_d = nc.dram_tensor("xT", [C, N], F32, kind="ExternalInput").ap()
    wqT_d = nc.dram_tensor("wqT", [C, 3 * C], F32, kind="ExternalInput").ap()
    wpT_d = nc.dram_tensor("wpT", [C, C], F32, kind="ExternalInput").ap()
    bias_d = nc.dram_tensor("bias_bc", [128, C], F32, kind="ExternalInput").ap()
    ones_d = nc.dram_tensor("ones_v", [128, H], F32, kind="ExternalInput").ap()
    onesr_d = nc.dram_tensor("ones_mask", [2, 128], F32, kind="ExternalInput").ap()
    out_d = nc.dram_tensor("out", [N, C], F32, kind="ExternalOutput").ap()

    qk_dt = BF16 if QK_BF16 else F32
    qk = (lambda ap: ap) if QK_BF16 else _r

    with tile.TileContext(nc) as tc:
        with (
            tc.tile_pool(name="big", bufs=1) as big,
            tc.tile_pool(name="wqk", bufs=8) as wqkp,
            tc.tile_pool(name="e", bufs=4) as ep,
            tc.tile_pool(name="outb", bufs=2) as outp,
            tc.tile_pool(name="norm", bufs=2) as normp,
            tc.tile_pool(name="psA", bufs=2, space="PSUM") as psA,
            tc.tile_pool(name="psC", bufs=2, space="PSUM") as psC,
        ):
            # ---- persistent SBUF tensors -------------------------------
            xqk = big.tile([128, CT, N], F32, name="xqk", tag="xqk")
            wvs = big.tile([128, CT, C], F32, name="wvs", tag="wvs")
            wps = big.tile([128, CT, C], BF16, name="wps", tag="wps")
            QT = big.tile([128, CT, N], qk_dt, name="QT", tag="QT")
            KT = big.tile([128, CT, N], qk_dt, name="KT", tag="KT")
            V = big.tile([128, NT, H * (D + 1)], BF16, name="V", tag="V")
            ctxN = big.tile([128, CT, N], BF16, name="ctxN", tag="ctxN")
            bias_sb = big.tile([128, C], F32, name="bias_sb", tag="bias")
            ones_mask = [
                big.tile([1, 128], F32, name=f"ones_mask{i}", tag=f"onesr{i}")
                for i in range(2)
            ]

            nc.sync.dma_start(bias_sb[:], bias_d[:])
            for i in range(2):
                nc.sync.dma_start(_r(ones_mask[i][:]), _r(onesr_d[i:i + 1, :]))
            for ct in range(CT):
                nc.sync.dma_start(
                    _r(xqk[:, ct, :]), _r(xT_d[ct * 128:(ct + 1) * 128, :])
                )
                nc.sync.dma_start(
                    _r(wvs[:, ct, :]), _r(wqT_d[ct * 128:(ct + 1) * 128, 2 * C:3 * C])
                )
            for nt in range(NT):
                vt = V[:, nt, :].rearrange("p (h e) -> p h e", e=D + 1)
                nc.gpsimd.dma_start(
                    vt[:, :, D:D + 1], ones_d.rearrange("p (h o) -> p h o", o=1)
                )

            # ---- phase B: V (token-major, bf16) ------------------------
            for nt in range(NT):
                pv = psA.tile([128, N], F32, tag="ps", name=f"pv{nt}")
                for ct in range(CT):
                    lhsT = xqk[:, ct, nt * 128:(nt + 1) * 128]
                    for lo, w in ((0, 512), (512, 256)):
                        nc.tensor.matmul(
                            pv[:, lo:lo + w],
                            _r(lhsT),
                            _r(wvs[:, ct, lo:lo + w]),
                            start=(ct == 0),
                            stop=(ct == CT - 1),
                        )
                vt = V[:, nt, :].rearrange("p (h e) -> p h e", e=D + 1)
                nc.scalar.copy(
                    vt[:, :, 0:D], pv[:, 0:C].rearrange("p (h d) -> p h d", d=D)
                )

            # ---- phase C: QT / KT (feature-major) ----------------------
            for jt in range(CT):
                for base, dst in ((0, QT), (C, KT)):
                    wts = []
                    for ct in range(CT):
                        wt = wqkp.tile(
                            [128, 128], qk_dt, tag="wqk", name=f"w{base}_{jt}_{ct}"
                        )
                        src = wqT_d[
                            ct * 128:(ct + 1) * 128,
                            base + jt * 128:base + (jt + 1) * 128,
                        ]
                        if QK_BF16:
                            nc.gpsimd.dma_start(wt[:], src)
                        else:
                            nc.sync.dma_start(_r(wt[:]), _r(src))
                        wts.append(wt)
                    ps = psA.tile([128, N], F32, tag="ps", name=f"q{base}_{jt}")
                    for ct in range(CT):
                        for qc in range(2):
                            nc.tensor.matmul(
                                ps[:, qc * 512:(qc + 1) * 512],
                                qk(wts[ct][:]),
                                qk(xqk[:, ct, qc * 512:(qc + 1) * 512]),
                                start=(ct == 0),
                                stop=(ct == CT - 1),
                            )
                    nc.scalar.copy(qk(dst[:, jt, :]), ps[:])

            # proj weights are first needed far later; load them now so the
            # casting DMAs do not delay the startup x/w loads
            for ct in range(CT):
                nc.gpsimd.dma_start(wps[:, ct, :], wpT_d[ct * 128:(ct + 1) * 128, :])

            # ---- phase D: attention, head pairs, row-packed S ----------
            deferred_norm = []

            def emit_norm(jobs):
                # jobs = halves of one or more pairs; per pair, build the
                # full [128, N] reciprocal-broadcast with two K=1 masked
                # ones-matmuls, then normalize with a single multiply.
                for i in range(0, len(jobs), 2):
                    emit_norm_pair(jobs[i:i + 2])

            def emit_norm_pair(jobs):
                p_ = jobs[0][2]
                rcrs = []
                for den_, h_, _p in jobs:
                    rc = normp.tile([1, N], F32, tag="rc", name=f"rc{h_}", bufs=2)
                    nc.vector.reciprocal_approx_fast(rc[:], den_[:])
                    rcr = normp.tile([1, N], F32, tag="rcr", name=f"rcr{h_}", bufs=2)
                    nc.scalar.copy(_r(rcr[:]), rc[:])
                    rcrs.append(rcr)
                bc_ps = psA.tile([128, N], F32, tag="ps", name=f"bcp{p_}")
                for qc in range(2):
                    for half, rcr in enumerate(rcrs):
                        nc.tensor.matmul(
                            bc_ps[:, qc * 512:(qc + 1) * 512],
                            _r(ones_mask[half][:]),
                            _r(rcr[:, qc * 512:(qc + 1) * 512]),
                            start=(half == 0),
                            stop=(half == len(rcrs) - 1),
                        )
                bc = normp.tile([128, N], F32, tag="bc", name=f"bc{p_}", bufs=1)
                nc.vector.tensor_copy(bc[:], bc_ps[:])
                nc.vector.tensor_mul(ctxN[:, p_, :], ctxN[:, p_, :], bc[:])

            # Software pipeline across head pairs: during pair p's S/exp
            # stream (ACT-paced), the PE executes pair p-1's PV matmuls,
            # whose E tiles are already complete. PV then never waits on the
            # in-flight exp, and attention runs at the ACT exp rate.
            def emit_pv(pcps, pes, pp, kt):
                for half in range(2):
                    h = 2 * pp + half
                    for qc in range(2):
                        nc.tensor.matmul(
                            pcps[half][:, qc * 512:(qc + 1) * 512],
                            V[:, kt, h * (D + 1):(h + 1) * (D + 1)],
                            pes[kt][half][:, qc * 512:(qc + 1) * 512],
                            start=(kt == 0),
                            stop=(kt == NT - 1),
                        )

            def emit_evac(pcps, pp):
                for half in range(2):
                    h = 2 * pp + half
                    po = half * 64
                    nc.vector.tensor_copy(
                        ctxN[po:po + 64, pp, :], pcps[half][0:D, :]
                    )
                    den = normp.tile([1, N], F32, tag="den", name=f"den{h}")
                    nc.scalar.copy(den[:], pcps[half][D:D + 1, :])
                    deferred_norm.append((den, h, pp))

            prev = None
            for p in range(CT):  # 6 head pairs; pair p = heads (2p, 2p+1)
                cps = [
                    psC.tile([D + 1, N], F32, tag="ctx", name=f"ctx{2 * p + i}")
                    for i in range(2)
                ]
                es = []
                for kt in range(NT):
                    sps = [
                        psA.tile([128, N], F32, tag="ps", name=f"s{2 * p + i}_{kt}")
                        for i in range(2)
                    ]
                    for half in range(2):
                        po = half * 64
                        for qc in range(2):
                            nc.tensor.matmul(
                                sps[half][:, qc * 512:(qc + 1) * 512],
                                qk(KT[po:po + 64, p, kt * 128:(kt + 1) * 128]),
                                qk(QT[po:po + 64, p, qc * 512:(qc + 1) * 512]),
                                start=True,
                                stop=True,
                                tile_position=(po, 0),
                            )
                    row = []
                    for half in range(2):
                        h = 2 * p + half
                        e = ep.tile(
                            [128, N], BF16, tag="e", name=f"e{h}_{kt}", bufs=12
                        )
                        nc.scalar.activation(e[:], sps[half][:], EXP, scale=SCALE)
                        row.append(e)
                    es.append(row)
                    if prev is not None:
                        emit_pv(prev[0], prev[1], prev[2], kt)
                    if kt == 1 and deferred_norm:
                        # normalize the pair before last while streams run
                        emit_norm(deferred_norm)
                        deferred_norm = []
                if prev is not None:
                    emit_evac(prev[0], prev[2])
                prev = (cps, es, p)
            # drain: PV + evac for the final pair
            for kt in range(NT):
                emit_pv(prev[0], prev[1], prev[2], kt)
            emit_evac(prev[0], prev[2])
            emit_norm(deferred_norm)
            deferred_norm = []

            # ---- phase E: output projection + bias ---------------------
            for nt in range(NT):
                ps = psA.tile([128, N], F32, tag="ps", name=f"po{nt}")
                for lo, w in ((0, 512), (512, 256)):
                    for ct in range(CT):
                        nc.tensor.matmul(
                            ps[:, lo:lo + w],
                            ctxN[:, ct, nt * 128:(nt + 1) * 128],
                            wps[:, ct, lo:lo + w],
                            start=(ct == 0),
                            stop=(ct == CT - 1),
                        )
                ob = outp.tile([128, C], F32, tag="ob", name=f"ob{nt}")
                nc.vector.tensor_add(ob[:], ps[:, 0:C], bias_sb[:])
                nc.sync.dma_start(out_d[nt * 128:(nt + 1) * 128, :], ob[:])

    nc.compile()
    return nc


_CACHE = {}


def _get_nc():
    if "nc" not in _CACHE:
        _CACHE["nc"] = build()
    return _CACHE["nc"]


def run(inputs, trace=False):
    """Run on hardware; returns (full_output [8,1024,768] f32, BassKernelResults)."""
    nc = _get_nc()
    x = np.asarray(inputs["x"], dtype=np.float32)
    w_qkv = np.asarray(inputs["w_qkv"], dtype=np.float32)
    w_proj = np.asarray(inputs["w_proj"], dtype=np.float32)
    b_proj = np.asarray(inputs["b_proj"], dtype=np.float32)

    xT = np.ascontiguousarray(x.transpose(0, 2, 1))          # [8, 768, 1024]
    wqT = np.ascontiguousarray(w_qkv.T)                       # [768, 2304]
    wpT = np.ascontiguousarray(w_proj.T)                      # [768, 768]
    bias_bc = np.ascontiguousarray(np.broadcast_to(b_proj.reshape(1, C), (128, C)))
    ones_v = np.ones((128, H), dtype=np.float32)

    in_maps = [
        {
            "xT": xT[b],
            "wqT": wqT,
            "wpT": wpT,
            "bias_bc": bias_bc,
            "ones_v": ones_v,
            "ones_mask": np.kron(np.eye(2), np.ones((1, 64))).astype(np.float32),
        }
        for b in range(N_CORES)
    ]
    res = run_bass_kernel_spmd(nc, in_maps, list(range(N_CORES)), trace=trace)
    out = np.stack([res.results[b]["out"] for b in range(N_CORES)])
    return out, res


def kernel(x, w_qkv, w_proj, b_proj):
    out, _ = run(
        {"x": x, "w_qkv": w_qkv, "w_proj": w_proj, "b_proj": b_proj}, trace=False
    )
    return out
